# revision 13
# baseline (speedup 1.0000x reference)
"""Trainium2 Bass kernel for nn_Attention_9612136808713.

Transformer-XL style attention (rel-shift pos bias, causal, 16 heads),
b=2, n=2048, dim=1024. Sharded over 8 NeuronCores: data-parallel on
batch (2) x tensor-parallel on heads (4 groups of 4 heads). Wq/Wkv
column-split per head group; Wo row-split with the partial-sum
reduction done on the host during unsharding.

Design notes (v2):
- Scores are computed TRANSPOSED (S^T[j, i] = k_j . q_i) so the exp
  output is directly P^T, ready as the stationary operand of attn@v —
  no per-block PE transposes of the probability matrix.
- The rel-shift position bias U[i, r] = q_i . p_r is written to a
  row-padded DRAM scratch (rows of length N+128 whose last 128 cols
  are NEG-filled), and read back SHIFTED + TRANSPOSED in one XBAR
  DMA-transpose instruction per (head, j-block). The NEG pad makes
  the causal mask automatic after exp.
- attn@v uses a ones-column appended to v so the softmax denominator
  accumulates in the same PSUM tile as the numerator.
- Everything is bf16 on the wire; fp32 only in PSUM.
- p projection (pos_emb @ Wp + bp) is precomputed on the host.

Self-contained: only needs numpy + the concourse/bass toolchain.
"""

import contextlib
import json

import numpy as np

import concourse.bass as bass
import concourse.mybir as mybir
import concourse.tile as tile
from concourse.bass_utils import run_bass_kernel_spmd

F32 = mybir.dt.float32
BF16 = mybir.dt.bfloat16

N = 2048
DIM = 1024
HEADS = 16
D = 64          # head dim
HPC = 4         # heads per core
PAIRS = 2       # head pairs per core
CH = 512        # free-dim chunk (one PSUM bank of fp32)
NB = N // 128   # 16 blocks
KC = DIM // 128  # 8 contraction chunks
W = N + 128     # padded UB row length
SCALE = D ** -0.5
NEG = -30000.0  # exp(NEG) == 0, no inf/nan hazards


# --------------------------------------------------------------------------
# Wait-splitting post-pass: this container's walrus build accepts only ONE
# sync-wait command per instruction, while Tile attaches several. Splitting
# an AND-wait into single-wait NoOps on the same engine immediately before
# the instruction is semantically equivalent (sem-ge waits are monotonic).
# --------------------------------------------------------------------------

def _split_waits_json_bytes(raw: bytes) -> bytes:
    d = json.loads(raw)
    counter = [0]

    def fix_block(b):
        out = []
        for inst in b.get("instructions", []):
            si = inst.get("sync_info")
            waits = (si or {}).get("on_wait") or []
            if len(waits) > 1:
                eng = inst.get("engine")
                for w in waits[:-1]:
                    counter[0] += 1
                    out.append(
                        {
                            "engine": eng,
                            "ins": [],
                            "outs": [],
                            "name": f"WSPLIT-{counter[0]}",
                            "opcode": "NoOp",
                            "sync_info": {"on_update": [], "on_wait": [w]},
                        }
                    )
                si["on_wait"] = [waits[-1]]
            out.append(inst)
        b["instructions"] = out

    for f in d.get("functions", []):
        for b in f.get("blocks", []):
            fix_block(b)
    return json.dumps(d).encode()


def _patch_bass(nc):
    orig = nc.to_json_bytes

    def patched():
        return _split_waits_json_bytes(orig())

    nc.to_json_bytes = patched
    return nc


def build_nc():
    nc = bass.Bass()

    xT = nc.dram_tensor("xT", [DIM, N], BF16, kind="ExternalInput")
    wq = nc.dram_tensor("wq", [DIM, 256], BF16, kind="ExternalInput")
    wk = nc.dram_tensor("wk", [DIM, 256], BF16, kind="ExternalInput")
    wv = nc.dram_tensor("wv", [DIM, 256], BF16, kind="ExternalInput")
    wo = nc.dram_tensor("wo", [256, DIM], BF16, kind="ExternalInput")
    pT = nc.dram_tensor("pT", [128, N], BF16, kind="ExternalInput")
    bq = nc.dram_tensor("bq", [256, 1], F32, kind="ExternalInput")
    bks = nc.dram_tensor("bks", [256, 1], F32, kind="ExternalInput")  # SCALE*bk
    bvb = nc.dram_tensor("bvb", [128, 256], F32, kind="ExternalInput")
    ident = nc.dram_tensor("ident", [128, 128], BF16, kind="ExternalInput")
    out = nc.dram_tensor("out", [N, DIM], BF16, kind="ExternalOutput")

    # pos-bias scratch: per head, N rows of length W (last 128 cols = NEG pad)
    UB = nc.dram_tensor("UB", [HPC * N * W], BF16)
    # attn-out roundtrip scratch for the XBAR transpose (per pair)
    ADR = nc.dram_tensor("ADR", [PAIRS * NB * 128 * 128], BF16)

    with tile.TileContext(nc) as tc:
        with contextlib.ExitStack() as ctx:
            const = ctx.enter_context(tc.tile_pool(name="const", bufs=1))
            pers = ctx.enter_context(tc.tile_pool(name="pers", bufs=1))

            # ---- constants -------------------------------------------------
            ident_sb = const.tile([128, 128], BF16, tag="ident")
            nc.sync.dma_start(out=ident_sb, in_=ident[:, :])
            neg_sb = const.tile([128, 128], BF16, tag="negs")
            nc.vector.memset(neg_sb, NEG)

            # ---- persistent activations -----------------------------------
            qT = [pers.tile([128, N], BF16, tag=f"qT{p}", name=f"qT{p}") for p in range(PAIRS)]
            kT = [pers.tile([128, N], BF16, tag=f"kT{p}", name=f"kT{p}") for p in range(PAIRS)]
            pT_sb = pers.tile([128, N], BF16, tag="pT")
            nc.scalar.dma_start(out=pT_sb, in_=pT[:, :])
            # v with a ones column per head: [j-part, jb, head-slot, 65]
            v_sb = pers.tile([128, NB, HPC, 65], BF16, tag="v")
            nc.gpsimd.memset(v_sb[:, :, :, 64:65], 1.0)
            wo_sb = [pers.tile([128, DIM], BF16, tag=f"wo{p}", name=f"wo{p}") for p in range(PAIRS)]
            for p in range(PAIRS):
                nc.sync.dma_start(out=wo_sb[p], in_=wo[128 * p:128 * p + 128, :])
            aT = [pers.tile([128, N], BF16, tag=f"aT{p}", name=f"aT{p}") for p in range(PAIRS)]
            A_sb = [pers.tile([128, NB, 2, D], BF16, tag=f"Asb{p}", name=f"Asb{p}") for p in range(PAIRS)]

            # ---- neg-fill the UB pad bands (mask source) ------------------
            for h in range(HPC):
                dst = bass.AP(tensor=UB, offset=h * N * W + N, ap=[[W, N], [1, 128]])
                srcb = bass.AP(
                    tensor=neg_sb.tensor,
                    offset=neg_sb.offset,
                    ap=[neg_sb.ap[0], [0, NB], [1, 128]],
                )
                nc.gpsimd.dma_start(out=dst, in_=srcb)

            # ---- phase A: projections -------------------------------------
            with contextlib.ExitStack() as sA:
                pp = sA.enter_context(tc.tile_pool(name="apsum", bufs=1, space="PSUM"))
                stream = sA.enter_context(tc.tile_pool(name="xstream", bufs=1))
                wq_sb = stream.tile([128, KC, 256], BF16, tag="wq")
                wk_sb = stream.tile([128, KC, 256], BF16, tag="wk")
                wv_sb = stream.tile([128, KC, 256], BF16, tag="wv")
                nc.sync.dma_start(out=wq_sb, in_=wq[:, :].rearrange("(kc p) m -> p kc m", p=128))
                nc.sync.dma_start(out=wk_sb, in_=wk[:, :].rearrange("(kc p) m -> p kc m", p=128))
                nc.sync.dma_start(out=wv_sb, in_=wv[:, :].rearrange("(kc p) m -> p kc m", p=128))
                bq_sb = stream.tile([128, PAIRS], F32, tag="bq")
                bk_sb = stream.tile([128, PAIRS], F32, tag="bk")
                for p in range(PAIRS):
                    nc.sync.dma_start(out=bq_sb[:, p:p + 1], in_=bq[128 * p:128 * p + 128, :])
                    nc.sync.dma_start(out=bk_sb[:, p:p + 1], in_=bks[128 * p:128 * p + 128, :])
                bvb_sb = stream.tile([128, 256], F32, tag="bvb")
                nc.sync.dma_start(out=bvb_sb, in_=bvb[:, :])
                x_t = []
                for kc in range(KC):
                    t = stream.tile([128, N], BF16, tag=f"xt{kc}")
                    q_eng = nc.gpsimd if kc % 2 == 0 else nc.scalar
                    q_eng.dma_start(out=t, in_=xT[128 * kc:128 * kc + 128, :])
                    x_t.append(t)

                # q^T / k^T (pair-packed: [128 = headA d | headB d, N])
                for p in range(PAIRS):
                    for qk in range(2):
                        pss = [pp.tile([128, CH], F32, tag=f"ps{c}", name=f"pqk{qk}_{p}_{c}")
                               for c in range(N // CH)]
                        w_sb = wq_sb if qk == 0 else wk_sb
                        for kc in range(KC):
                            for c in range(N // CH):
                                nc.tensor.matmul(
                                    pss[c], w_sb[:, kc, 128 * p:128 * p + 128],
                                    x_t[kc][:, CH * c:CH * c + CH],
                                    start=(kc == 0), stop=(kc == KC - 1),
                                )
                        for c in range(N // CH):
                            nc.scalar.activation(
                                out=(qT if qk == 0 else kT)[p][:, CH * c:CH * c + CH],
                                in_=pss[c],
                                func=mybir.ActivationFunctionType.Identity,
                                bias=(bq_sb if qk == 0 else bk_sb)[:, p:p + 1],
                                scale=(1.0 if qk == 0 else SCALE),
                            )
                # v (natural layout), 4 j-blocks per pass
                for grp in range(4):
                    psvs = [pp.tile([128, 256], F32, tag=f"ps{j}", name=f"psv{grp}_{j}")
                            for j in range(4)]
                    for kc in range(KC):
                        for j in range(4):
                            jb = 4 * grp + j
                            nc.tensor.matmul(
                                psvs[j], x_t[kc][:, 128 * jb:128 * jb + 128],
                                wv_sb[:, kc, :],
                                start=(kc == 0), stop=(kc == KC - 1),
                            )
                    for j in range(4):
                        jb = 4 * grp + j
                        nc.vector.tensor_add(
                            out=v_sb[:, jb, :, 0:64],
                            in0=psvs[j][:, :].rearrange("p (h d) -> p h d", h=4),
                            in1=bvb_sb[:, :].rearrange("p (h d) -> p h d", h=4),
                        )

            # ---- phases B+C, software-pipelined by head -------------------
            with contextlib.ExitStack() as sBC:
                upp = sBC.enter_context(tc.tile_pool(name="upsum", bufs=1, space="PSUM"))
                spp = sBC.enter_context(tc.tile_pool(name="spsum", bufs=1, space="PSUM"))
                app = sBC.enter_context(tc.tile_pool(name="apsum2", bufs=1, space="PSUM"))
                ust = sBC.enter_context(tc.tile_pool(name="ustage", bufs=3))
                t2p = sBC.enter_context(tc.tile_pool(name="t2pool", bufs=2))
                ptp = sBC.enter_context(tc.tile_pool(name="ptpool", bufs=1))
                nrm = sBC.enter_context(tc.tile_pool(name="nrm", bufs=2))

                ccount = [0]

                def phase_B(h):
                    """U[i, r] = q_i . p_r for r in [N-128-i0, N); write to UB."""
                    p, half = divmod(h, 2)
                    for I in range(NB):
                        i0 = 128 * I
                        r0 = N - 128 - i0
                        span = i0 + 128
                        ub_t = ust.tile([128, N], BF16, tag="ub", name=f"ub_{h}_{I}")
                        for ci in range(-(-span // CH)):
                            rc = r0 + CH * ci
                            wdt = min(CH, N - rc)
                            psu = upp.tile([128, CH], F32, tag=f"psu{ci % 2}",
                                           name=f"psu_{h}_{I}_{ci}")
                            nc.tensor.matmul(
                                psu[:, :wdt],
                                qT[p][D * half:D * half + D, i0:i0 + 128],
                                pT_sb[D * half:D * half + D, rc:rc + wdt],
                                start=True, stop=True,
                                tile_position=(D * half, 0),
                            )
                            oc = CH * ci
                            if ccount[0] % 3 == 2:
                                nc.scalar.activation(
                                    out=ub_t[:, oc:oc + wdt], in_=psu[:, :wdt],
                                    func=mybir.ActivationFunctionType.Copy,
                                )
                            else:
                                nc.vector.tensor_copy(out=ub_t[:, oc:oc + wdt], in_=psu[:, :wdt])
                            ccount[0] += 1
                        dst = bass.AP(
                            tensor=UB,
                            offset=h * N * W + i0 * W + r0,
                            ap=[[W, 128], [1, span]],
                        )
                        q_eng = nc.gpsimd if I % 2 == 0 else nc.sync
                        q_eng.dma_start(out=dst, in_=ub_t[:, :span])

                def phase_C(h):
                    p, half = divmod(h, 2)
                    # prefetch all shifted+transposed pos-bias tiles for this head
                    t2s = []
                    for J in range(NB):
                        j0 = 128 * J
                        span = N - j0
                        t2 = t2p.tile([128, span], BF16, tag=f"t2_{J}", name=f"t2_{h}_{J}")
                        src = bass.AP(
                            tensor=UB,
                            offset=h * N * W + j0 * (W - 1) + (N - 1) + j0,
                            ap=[[W - 1, span], [1, 128]],
                        )
                        nc.sync.dma_start(out=t2, in_=src, transpose=True)
                        t2s.append(t2)

                    rden = nrm.tile([128, NB], F32, tag="rden", name=f"rden_{h}")
                    for I4 in range(NB // 4):
                        iw0 = CH * I4
                        # one full PSUM bank per open A slot (4 slots)
                        abank = [app.tile([128, CH], F32, tag=f"ab{bk}",
                                          name=f"ab_{h}_{I4}_{bk}") for bk in range(4)]
                        for J in range(4 * I4 + 4):
                            j0 = 128 * J
                            ic = max(iw0, j0)
                            wdt = iw0 + CH - ic
                            pss = spp.tile([128, CH], F32, tag=f"pss{J % 2}",
                                           name=f"pss_{h}_{I4}_{J}")
                            nc.tensor.matmul(
                                pss[:, :wdt],
                                kT[p][D * half:D * half + D, j0:j0 + 128],
                                qT[p][D * half:D * half + D, ic:ic + wdt],
                                start=True, stop=False,
                                tile_position=(D * half, 0),
                                skip_group_check=True,
                            )
                            nc.tensor.matmul(
                                pss[:, :wdt], ident_sb,
                                t2s[J][:, ic - j0:ic - j0 + wdt],
                                start=False, stop=True,
                                skip_group_check=True,
                            )
                            P_t = ptp.tile([128, CH], BF16, tag=f"P{J % 3}",
                                           name=f"P_{h}_{I4}_{J}")
                            nc.scalar.activation(
                                out=P_t[:, :wdt], in_=pss[:, :wdt],
                                func=mybir.ActivationFunctionType.Exp,
                            )
                            for bk in range(4):
                                I = 4 * I4 + bk
                                if I < J:
                                    continue
                                off = 128 * I - ic
                                nc.tensor.matmul(
                                    abank[bk][:, :65],
                                    P_t[:, off:off + 128],
                                    v_sb[:, J, 2 * p + half, :],
                                    start=(J == 0),
                                    stop=(J == I),
                                    skip_group_check=True,
                                )
                        # normalize the 4 finished slots: A[:, :64] / A[:, 64]
                        for bk in range(4):
                            I = 4 * I4 + bk
                            nc.vector.reciprocal(
                                out=rden[:, I:I + 1], in_=abank[bk][:, 64:65]
                            )
                            if bk % 2 == 0:
                                nc.scalar.activation(
                                    out=A_sb[p][:, I, half, :],
                                    in_=abank[bk][:, :64],
                                    func=mybir.ActivationFunctionType.Identity,
                                    scale=rden[:, I:I + 1],
                                )
                            else:
                                nc.vector.tensor_scalar_mul(
                                    out=A_sb[p][:, I, half, :],
                                    in0=abank[bk][:, :64],
                                    scalar1=rden[:, I:I + 1],
                                )
                    # pair complete -> A_pair [i, d2] -> DRAM -> XBAR -> aT [d2, i]
                    if half == 1:
                        dst = bass.AP(
                            tensor=ADR,
                            offset=p * NB * 128 * 128,
                            ap=[[128, 128], [128 * 128, NB], [1, 128]],
                        )
                        nc.scalar.dma_start(out=dst, in_=A_sb[p])
                        for I in range(NB):
                            src = bass.AP(
                                tensor=ADR,
                                offset=(p * NB + I) * 128 * 128,
                                ap=[[128, 128], [1, 128]],
                            )
                            nc.scalar.dma_start(
                                out=aT[p][:, 128 * I:128 * I + 128],
                                in_=src,
                                transpose=True,
                            )

                phase_B(0)
                phase_B(1)
                phase_C(0)
                phase_B(2)
                phase_C(1)
                phase_B(3)
                phase_C(2)
                phase_C(3)

            # ---- phase D: out partial = A^T rows @ Wo ---------------------
            with contextlib.ExitStack() as sD:
                opp = sD.enter_context(tc.tile_pool(name="opsum", bufs=2, space="PSUM"))
                ost = sD.enter_context(tc.tile_pool(name="ostage", bufs=2))
                for Ip in range(NB // 2):
                    o2 = ost.tile([128, 2, DIM], BF16, tag="o2", name=f"o2_{Ip}")
                    for b2 in range(2):
                        I = 2 * Ip + b2
                        i0 = 128 * I
                        pso = opp.tile([128, DIM], F32, tag="pso", name=f"pso_{I}")
                        for c in range(DIM // CH):
                            for p in range(PAIRS):
                                nc.tensor.matmul(
                                    pso[:, CH * c:CH * c + CH],
                                    aT[p][:, i0:i0 + 128],
                                    wo_sb[p][:, CH * c:CH * c + CH],
                                    start=(p == 0), stop=(p == PAIRS - 1),
                                    skip_group_check=True,
                                )
                        if b2 == 0:
                            nc.vector.tensor_copy(out=o2[:, b2, :], in_=pso)
                        else:
                            nc.scalar.activation(
                                out=o2[:, b2, :], in_=pso,
                                func=mybir.ActivationFunctionType.Copy,
                            )
                    dst = bass.AP(
                        tensor=out,
                        offset=256 * Ip * DIM,
                        ap=[[DIM, 128], [128 * DIM, 2], [1, DIM]],
                    )
                    nc.scalar.dma_start(out=dst, in_=o2)

    _patch_bass(nc)
    return nc


_NC_CACHE = {}


def _get_nc():
    if "nc" not in _NC_CACHE:
        _NC_CACHE["nc"] = build_nc()
    return _NC_CACHE["nc"]


def _bf16(x):
    import ml_dtypes
    return np.asarray(x, dtype=ml_dtypes.bfloat16)


def kernel(x, pos_emb, Wq, bq, Wkv, bkv, Wp, bp, Wo, bo):
    x = np.asarray(x, dtype=np.float32)
    pos_emb = np.asarray(pos_emb, dtype=np.float32)
    Wq = np.asarray(Wq, dtype=np.float32)
    bq = np.asarray(bq, dtype=np.float32)
    Wkv = np.asarray(Wkv, dtype=np.float32)
    bkv = np.asarray(bkv, dtype=np.float32)
    Wp = np.asarray(Wp, dtype=np.float32)
    bp = np.asarray(bp, dtype=np.float32)
    Wo = np.asarray(Wo, dtype=np.float32)
    bo = np.asarray(bo, dtype=np.float32)

    b, n, dim = x.shape
    assert (b, n, dim) == (2, N, DIM)

    xTs = [_bf16(np.ascontiguousarray(x[bi].T)) for bi in range(b)]
    # host-side p projection: [n, d] -> scaled, transposed, duplicated
    p_proj = ((pos_emb @ Wp) + bp) * SCALE
    pT_np = _bf16(np.vstack([p_proj.T, p_proj.T]))

    ident = _bf16(np.eye(128))

    in_maps = []
    for c in range(8):
        bi, g = divmod(c, HPC)
        cols = slice(256 * g, 256 * g + 256)
        in_maps.append(
            {
                "xT": xTs[bi],
                "wq": _bf16(Wq[:, cols]),
                "wk": _bf16(Wkv[:, 256 * g:256 * g + 256]),
                "wv": _bf16(Wkv[:, DIM + 256 * g:DIM + 256 * g + 256]),
                "wo": _bf16(Wo[256 * g:256 * g + 256, :]),
                "pT": pT_np,
                "bq": np.ascontiguousarray(bq[cols])[:, None],
                "bks": (np.ascontiguousarray(bkv[256 * g:256 * g + 256]) * SCALE)[:, None],
                "bvb": np.broadcast_to(
                    bkv[DIM + 256 * g:DIM + 256 * g + 256], (128, 256)
                ).copy(),
                "ident": ident,
            }
        )

    nc = _get_nc()
    res = run_bass_kernel_spmd(nc, in_maps, core_ids=list(range(8)))

    outp = np.zeros((b, n, dim), dtype=np.float32)
    for c in range(8):
        bi = c // HPC
        outp[bi] += res.results[c]["out"].astype(np.float32)
    outp += bo
    return outp


# revision 19
# speedup vs baseline: 1.0565x; 1.0565x over previous
"""Trainium2 Bass kernel for nn_Attention_9612136808713.

Transformer-XL style attention (rel-shift pos bias, causal, 16 heads),
b=2, n=2048, dim=1024. Sharded over 8 NeuronCores: data-parallel on
batch (2) x tensor-parallel on heads (4 groups of 4 heads). Wq/Wkv
column-split per head group; Wo row-split with the partial-sum
reduction done on the host during unsharding.

Design notes (v2):
- Scores are computed TRANSPOSED (S^T[j, i] = k_j . q_i) so the exp
  output is directly P^T, ready as the stationary operand of attn@v —
  no per-block PE transposes of the probability matrix.
- The rel-shift position bias U[i, r] = q_i . p_r is written to a
  row-padded DRAM scratch (rows of length N+128 whose last 128 cols
  are NEG-filled), and read back SHIFTED + TRANSPOSED in one XBAR
  DMA-transpose instruction per (head, j-block). The NEG pad makes
  the causal mask automatic after exp.
- attn@v uses a ones-column appended to v so the softmax denominator
  accumulates in the same PSUM tile as the numerator.
- Everything is bf16 on the wire; fp32 only in PSUM.
- p projection (pos_emb @ Wp + bp) is precomputed on the host.

Self-contained: only needs numpy + the concourse/bass toolchain.
"""

import contextlib
import json

import numpy as np

import concourse.bass as bass
import concourse.mybir as mybir
import concourse.tile as tile
from concourse.bass_utils import run_bass_kernel_spmd

F32 = mybir.dt.float32
BF16 = mybir.dt.bfloat16

N = 2048
DIM = 1024
HEADS = 16
D = 64          # head dim
HPC = 4         # heads per core
PAIRS = 2       # head pairs per core
CH = 512        # free-dim chunk (one PSUM bank of fp32)
NB = N // 128   # 16 blocks
KC = DIM // 128  # 8 contraction chunks
W = N + 128     # padded UB row length
SCALE = D ** -0.5
NEG = -30000.0  # exp(NEG) == 0, no inf/nan hazards


# --------------------------------------------------------------------------
# Wait-splitting post-pass: this container's walrus build accepts only ONE
# sync-wait command per instruction, while Tile attaches several. Splitting
# an AND-wait into single-wait NoOps on the same engine immediately before
# the instruction is semantically equivalent (sem-ge waits are monotonic).
# --------------------------------------------------------------------------

def _split_waits_json_bytes(raw: bytes) -> bytes:
    d = json.loads(raw)
    counter = [0]

    def fix_block(b):
        out = []
        for inst in b.get("instructions", []):
            si = inst.get("sync_info")
            waits = (si or {}).get("on_wait") or []
            if len(waits) > 1:
                eng = inst.get("engine")
                for w in waits[:-1]:
                    counter[0] += 1
                    out.append(
                        {
                            "engine": eng,
                            "ins": [],
                            "outs": [],
                            "name": f"WSPLIT-{counter[0]}",
                            "opcode": "NoOp",
                            "sync_info": {"on_update": [], "on_wait": [w]},
                        }
                    )
                si["on_wait"] = [waits[-1]]
            out.append(inst)
        b["instructions"] = out

    for f in d.get("functions", []):
        for b in f.get("blocks", []):
            fix_block(b)
    return json.dumps(d).encode()


def _patch_bass(nc):
    orig = nc.to_json_bytes

    def patched():
        return _split_waits_json_bytes(orig())

    nc.to_json_bytes = patched
    return nc


def build_nc():
    nc = bass.Bass()

    xT = nc.dram_tensor("xT", [DIM, N], BF16, kind="ExternalInput")
    wq = nc.dram_tensor("wq", [DIM, 256], BF16, kind="ExternalInput")
    wk = nc.dram_tensor("wk", [DIM, 256], BF16, kind="ExternalInput")
    wv = nc.dram_tensor("wv", [DIM, 256], BF16, kind="ExternalInput")
    wo = nc.dram_tensor("wo", [256, DIM], BF16, kind="ExternalInput")
    pT = nc.dram_tensor("pT", [128, N], BF16, kind="ExternalInput")
    bq = nc.dram_tensor("bq", [256, 1], F32, kind="ExternalInput")
    bks = nc.dram_tensor("bks", [256, 1], F32, kind="ExternalInput")  # SCALE*bk
    bvb = nc.dram_tensor("bvb", [128, 256], F32, kind="ExternalInput")
    ident = nc.dram_tensor("ident", [128, 128], BF16, kind="ExternalInput")
    out = nc.dram_tensor("out", [N, DIM], BF16, kind="ExternalOutput")

    # pos-bias scratch: per head, N rows of length W (last 128 cols = NEG pad)
    UB = nc.dram_tensor("UB", [HPC * N * W], BF16)

    with tile.TileContext(nc) as tc:
        with contextlib.ExitStack() as ctx:
            const = ctx.enter_context(tc.tile_pool(name="const", bufs=1))
            pers = ctx.enter_context(tc.tile_pool(name="pers", bufs=1))

            # ---- persistent tiles -----------------------------------------
            ident_sb = const.tile([128, 128], BF16, tag="ident")
            neg_sb = const.tile([128, 128], BF16, tag="negs")
            qT = [pers.tile([128, N], BF16, tag=f"qT{p}", name=f"qT{p}") for p in range(PAIRS)]
            kT = [pers.tile([128, N], BF16, tag=f"kT{p}", name=f"kT{p}") for p in range(PAIRS)]
            pT_sb = pers.tile([128, N], BF16, tag="pT")
            # v with a ones column per head: [j-part, jb, head-slot, 65]
            v_sb = pers.tile([128, NB, HPC, 65], BF16, tag="v")
            wo_sb = [pers.tile([128, DIM], BF16, tag=f"wo{p}", name=f"wo{p}") for p in range(PAIRS)]
            aT = [pers.tile([128, N], BF16, tag=f"aT{p}", name=f"aT{p}") for p in range(PAIRS)]
            A_sb = [pers.tile([128, NB, 2, D], BF16, tag=f"Asb{p}", name=f"Asb{p}") for p in range(PAIRS)]

            # ---- phase A: projections -------------------------------------
            with contextlib.ExitStack() as sA:
                pp = sA.enter_context(tc.tile_pool(name="apsum", bufs=1, space="PSUM"))
                stream = sA.enter_context(tc.tile_pool(name="xstream", bufs=1))
                wq_sb = stream.tile([128, KC, 256], BF16, tag="wq")
                wk_sb = stream.tile([128, KC, 256], BF16, tag="wk")
                wv_sb = stream.tile([128, KC, 256], BF16, tag="wv")
                bq_sb = stream.tile([128, PAIRS], F32, tag="bq")
                bk_sb = stream.tile([128, PAIRS], F32, tag="bk")
                bvb_sb = stream.tile([128, 256], F32, tag="bvb")
                x_t = [stream.tile([128, N], BF16, tag=f"xt{kc}", name=f"xt{kc}")
                       for kc in range(KC)]
                # critical path first: wq then x chunks, split across queues
                nc.sync.dma_start(out=wq_sb, in_=wq[:, :].rearrange("(kc p) m -> p kc m", p=128))
                nc.scalar.dma_start(out=x_t[0], in_=xT[0:128, :])
                for kc in range(1, KC):
                    q_eng = nc.sync if kc % 2 == 1 else nc.scalar
                    q_eng.dma_start(out=x_t[kc], in_=xT[128 * kc:128 * kc + 128, :])
                for p in range(PAIRS):
                    nc.scalar.dma_start(out=bq_sb[:, p:p + 1], in_=bq[128 * p:128 * p + 128, :])
                    nc.scalar.dma_start(out=bk_sb[:, p:p + 1], in_=bks[128 * p:128 * p + 128, :])
                nc.sync.dma_start(out=wk_sb, in_=wk[:, :].rearrange("(kc p) m -> p kc m", p=128))
                nc.sync.dma_start(out=wv_sb, in_=wv[:, :].rearrange("(kc p) m -> p kc m", p=128))
                nc.scalar.dma_start(out=bvb_sb, in_=bvb[:, :])
                # deferred loads/fills (needed in later phases)
                nc.gpsimd.dma_start(out=ident_sb, in_=ident[:, :])
                nc.vector.memset(neg_sb, NEG)
                nc.gpsimd.memset(v_sb[:, :, :, 64:65], 1.0)
                nc.scalar.dma_start(out=pT_sb, in_=pT[:, :])
                for p in range(PAIRS):
                    nc.sync.dma_start(out=wo_sb[p], in_=wo[128 * p:128 * p + 128, :])
                for h in range(HPC):
                    dst = bass.AP(tensor=UB, offset=h * N * W + N, ap=[[W, N], [1, 128]])
                    srcb = bass.AP(
                        tensor=neg_sb.tensor,
                        offset=neg_sb.offset,
                        ap=[neg_sb.ap[0], [0, NB], [1, 128]],
                    )
                    nc.gpsimd.dma_start(out=dst, in_=srcb)

                # q^T / k^T (pair-packed: [128 = headA d | headB d, N])
                for p in range(PAIRS):
                    for qk in range(2):
                        pss = [pp.tile([128, CH], F32, tag=f"ps{c}", name=f"pqk{qk}_{p}_{c}")
                               for c in range(N // CH)]
                        w_sb = wq_sb if qk == 0 else wk_sb
                        for kc in range(KC):
                            for c in range(N // CH):
                                nc.tensor.matmul(
                                    pss[c], w_sb[:, kc, 128 * p:128 * p + 128],
                                    x_t[kc][:, CH * c:CH * c + CH],
                                    start=(kc == 0), stop=(kc == KC - 1),
                                )
                        for c in range(N // CH):
                            nc.scalar.activation(
                                out=(qT if qk == 0 else kT)[p][:, CH * c:CH * c + CH],
                                in_=pss[c],
                                func=mybir.ActivationFunctionType.Identity,
                                bias=(bq_sb if qk == 0 else bk_sb)[:, p:p + 1],
                                scale=(1.0 if qk == 0 else SCALE),
                            )
                # v (natural layout), 4 j-blocks per pass
                for grp in range(4):
                    psvs = [pp.tile([128, 256], F32, tag=f"ps{j}", name=f"psv{grp}_{j}")
                            for j in range(4)]
                    for kc in range(KC):
                        for j in range(4):
                            jb = 4 * grp + j
                            nc.tensor.matmul(
                                psvs[j], x_t[kc][:, 128 * jb:128 * jb + 128],
                                wv_sb[:, kc, :],
                                start=(kc == 0), stop=(kc == KC - 1),
                            )
                    for j in range(4):
                        jb = 4 * grp + j
                        nc.vector.tensor_add(
                            out=v_sb[:, jb, :, 0:64],
                            in0=psvs[j][:, :].rearrange("p (h d) -> p h d", h=4),
                            in1=bvb_sb[:, :].rearrange("p (h d) -> p h d", h=4),
                        )

            # ---- phases B+C, software-pipelined by head -------------------
            with contextlib.ExitStack() as sBC:
                upp = sBC.enter_context(tc.tile_pool(name="upsum", bufs=1, space="PSUM"))
                spp = sBC.enter_context(tc.tile_pool(name="spsum", bufs=1, space="PSUM"))
                app = sBC.enter_context(tc.tile_pool(name="apsum2", bufs=1, space="PSUM"))
                ust = sBC.enter_context(tc.tile_pool(name="ustage", bufs=3))
                t2p = sBC.enter_context(tc.tile_pool(name="t2pool", bufs=2))
                ptp = sBC.enter_context(tc.tile_pool(name="ptpool", bufs=1))
                nrm = sBC.enter_context(tc.tile_pool(name="nrm", bufs=2))

                ccount = [0]

                def phase_B(h):
                    """U[i, r] = q_i . p_r for r in [N-128-i0, N); write to UB."""
                    p, half = divmod(h, 2)
                    for I in range(NB):
                        i0 = 128 * I
                        r0 = N - 128 - i0
                        span = i0 + 128
                        ub_t = ust.tile([128, N], BF16, tag="ub", name=f"ub_{h}_{I}")
                        for ci in range(-(-span // CH)):
                            rc = r0 + CH * ci
                            wdt = min(CH, N - rc)
                            psu = upp.tile([128, CH], F32, tag=f"psu{ci % 2}",
                                           name=f"psu_{h}_{I}_{ci}")
                            nc.tensor.matmul(
                                psu[:, :wdt],
                                qT[p][D * half:D * half + D, i0:i0 + 128],
                                pT_sb[D * half:D * half + D, rc:rc + wdt],
                                start=True, stop=True,
                                tile_position=(D * half, 0),
                            )
                            oc = CH * ci
                            if ccount[0] % 11 == 10:
                                nc.scalar.activation(
                                    out=ub_t[:, oc:oc + wdt], in_=psu[:, :wdt],
                                    func=mybir.ActivationFunctionType.Copy,
                                )
                            else:
                                nc.vector.tensor_copy(out=ub_t[:, oc:oc + wdt], in_=psu[:, :wdt])
                            ccount[0] += 1
                        dst = bass.AP(
                            tensor=UB,
                            offset=h * N * W + i0 * W + r0,
                            ap=[[W, 128], [1, span]],
                        )
                        q_eng = nc.gpsimd if I % 2 == 0 else nc.sync
                        q_eng.dma_start(out=dst, in_=ub_t[:, :span])

                def phase_C(h):
                    p, half = divmod(h, 2)
                    # prefetch all shifted+transposed pos-bias tiles for this head
                    t2s = []
                    for J in range(NB):
                        j0 = 128 * J
                        span = N - j0
                        t2 = t2p.tile([128, span], BF16, tag=f"t2_{J}", name=f"t2_{h}_{J}")
                        src = bass.AP(
                            tensor=UB,
                            offset=h * N * W + j0 * (W - 1) + (N - 1) + j0,
                            ap=[[W - 1, span], [1, 128]],
                        )
                        nc.sync.dma_start(out=t2, in_=src, transpose=True)
                        t2s.append(t2)

                    rden = nrm.tile([128, NB], F32, tag="rden", name=f"rden_{h}")

                    def normalize(I4, bk, abank):
                        """Drain one finished A slot: A[:, :64] / A[:, 64]."""
                        I = 4 * I4 + bk
                        nc.vector.reciprocal(
                            out=rden[:, I:I + 1], in_=abank[bk][:, 64:65]
                        )
                        nc.vector.tensor_scalar_mul(
                            out=A_sb[p][:, I, half, :],
                            in0=abank[bk][:, :64],
                            scalar1=rden[:, I:I + 1],
                        )

                    for I4 in range(NB // 4):
                        iw0 = CH * I4
                        # one full PSUM bank per open A slot (4 slots)
                        abank = [app.tile([128, CH], F32, tag=f"ab{bk}",
                                          name=f"ab_{h}_{I4}_{bk}") for bk in range(4)]
                        for J in range(4 * I4 + 4):
                            j0 = 128 * J
                            ic = max(iw0, j0)
                            wdt = iw0 + CH - ic
                            pss = spp.tile([128, CH], F32, tag=f"pss{J % 2}",
                                           name=f"pss_{h}_{I4}_{J}")
                            nc.tensor.matmul(
                                pss[:, :wdt],
                                kT[p][D * half:D * half + D, j0:j0 + 128],
                                qT[p][D * half:D * half + D, ic:ic + wdt],
                                start=True, stop=False,
                                tile_position=(D * half, 0),
                                skip_group_check=True,
                            )
                            nc.tensor.matmul(
                                pss[:, :wdt], ident_sb,
                                t2s[J][:, ic - j0:ic - j0 + wdt],
                                start=False, stop=True,
                                skip_group_check=True,
                            )
                            P_t = ptp.tile([128, CH], BF16, tag=f"P{J % 3}",
                                           name=f"P_{h}_{I4}_{J}")
                            nc.scalar.activation(
                                out=P_t[:, :wdt], in_=pss[:, :wdt],
                                func=mybir.ActivationFunctionType.Exp,
                            )
                            for bk in range(4):
                                I = 4 * I4 + bk
                                if I < J:
                                    continue
                                off = 128 * I - ic
                                nc.tensor.matmul(
                                    abank[bk][:, :65],
                                    P_t[:, off:off + 128],
                                    v_sb[:, J, 2 * p + half, :],
                                    start=(J == 0),
                                    stop=(J == I),
                                    skip_group_check=True,
                                )
                            # slot (J - 4*I4) just stopped at J == I: drain it
                            # now so its bank frees while later J's compute
                            if J >= 4 * I4:
                                normalize(I4, J - 4 * I4, abank)

                phase_B(0)
                phase_B(1)
                phase_C(0)
                phase_B(2)
                phase_C(1)
                phase_B(3)
                phase_C(2)
                phase_C(3)

            # ---- phase D: out partial = A^T rows @ Wo ---------------------
            with contextlib.ExitStack() as sD:
                opp = sD.enter_context(tc.tile_pool(name="opsum", bufs=2, space="PSUM"))
                tpp = sD.enter_context(tc.tile_pool(name="tpsum", bufs=2, space="PSUM"))
                ost = sD.enter_context(tc.tile_pool(name="ostage", bufs=2))
                # A [i, d2] -> aT [d2, i] via PE transposes
                for p in range(PAIRS):
                    for Ip in range(NB // 2):
                        pst = tpp.tile([128, 2, 128], BF16, tag="pst",
                                       name=f"pst_{p}_{Ip}")
                        for b2 in range(2):
                            I = 2 * Ip + b2
                            nc.tensor.transpose(
                                pst[:, b2, :], A_sb[p][:, I, :, :], ident_sb
                            )
                        if Ip % 2 == 0:
                            nc.vector.tensor_copy(
                                out=aT[p][:, 256 * Ip:256 * Ip + 256],
                                in_=pst[:, :, :],
                            )
                        else:
                            nc.scalar.activation(
                                out=aT[p][:, 256 * Ip:256 * Ip + 256],
                                in_=pst[:, :, :],
                                func=mybir.ActivationFunctionType.Copy,
                            )
                for Ip in range(NB // 2):
                    o2 = ost.tile([128, 2, DIM], BF16, tag="o2", name=f"o2_{Ip}")
                    for b2 in range(2):
                        I = 2 * Ip + b2
                        i0 = 128 * I
                        pso = opp.tile([128, DIM], F32, tag="pso", name=f"pso_{I}")
                        for c in range(DIM // CH):
                            for p in range(PAIRS):
                                nc.tensor.matmul(
                                    pso[:, CH * c:CH * c + CH],
                                    aT[p][:, i0:i0 + 128],
                                    wo_sb[p][:, CH * c:CH * c + CH],
                                    start=(p == 0), stop=(p == PAIRS - 1),
                                    skip_group_check=True,
                                )
                        if b2 == 0:
                            nc.vector.tensor_copy(out=o2[:, b2, :], in_=pso)
                        else:
                            nc.scalar.activation(
                                out=o2[:, b2, :], in_=pso,
                                func=mybir.ActivationFunctionType.Copy,
                            )
                    dst = bass.AP(
                        tensor=out,
                        offset=256 * Ip * DIM,
                        ap=[[DIM, 128], [128 * DIM, 2], [1, DIM]],
                    )
                    nc.scalar.dma_start(out=dst, in_=o2)

    _patch_bass(nc)
    return nc


_NC_CACHE = {}


def _get_nc():
    if "nc" not in _NC_CACHE:
        _NC_CACHE["nc"] = build_nc()
    return _NC_CACHE["nc"]


def _bf16(x):
    import ml_dtypes
    return np.asarray(x, dtype=ml_dtypes.bfloat16)


def kernel(x, pos_emb, Wq, bq, Wkv, bkv, Wp, bp, Wo, bo):
    x = np.asarray(x, dtype=np.float32)
    pos_emb = np.asarray(pos_emb, dtype=np.float32)
    Wq = np.asarray(Wq, dtype=np.float32)
    bq = np.asarray(bq, dtype=np.float32)
    Wkv = np.asarray(Wkv, dtype=np.float32)
    bkv = np.asarray(bkv, dtype=np.float32)
    Wp = np.asarray(Wp, dtype=np.float32)
    bp = np.asarray(bp, dtype=np.float32)
    Wo = np.asarray(Wo, dtype=np.float32)
    bo = np.asarray(bo, dtype=np.float32)

    b, n, dim = x.shape
    assert (b, n, dim) == (2, N, DIM)

    xTs = [_bf16(np.ascontiguousarray(x[bi].T)) for bi in range(b)]
    # host-side p projection: [n, d] -> scaled, transposed, duplicated
    p_proj = ((pos_emb @ Wp) + bp) * SCALE
    pT_np = _bf16(np.vstack([p_proj.T, p_proj.T]))

    ident = _bf16(np.eye(128))

    in_maps = []
    for c in range(8):
        bi, g = divmod(c, HPC)
        cols = slice(256 * g, 256 * g + 256)
        in_maps.append(
            {
                "xT": xTs[bi],
                "wq": _bf16(Wq[:, cols]),
                "wk": _bf16(Wkv[:, 256 * g:256 * g + 256]),
                "wv": _bf16(Wkv[:, DIM + 256 * g:DIM + 256 * g + 256]),
                "wo": _bf16(Wo[256 * g:256 * g + 256, :]),
                "pT": pT_np,
                "bq": np.ascontiguousarray(bq[cols])[:, None],
                "bks": (np.ascontiguousarray(bkv[256 * g:256 * g + 256]) * SCALE)[:, None],
                "bvb": np.broadcast_to(
                    bkv[DIM + 256 * g:DIM + 256 * g + 256], (128, 256)
                ).copy(),
                "ident": ident,
            }
        )

    nc = _get_nc()
    res = run_bass_kernel_spmd(nc, in_maps, core_ids=list(range(8)))

    outp = np.zeros((b, n, dim), dtype=np.float32)
    for c in range(8):
        bi = c // HPC
        outp[bi] += res.results[c]["out"].astype(np.float32)
    outp += bo
    return outp
